# revision 17
# baseline (speedup 1.0000x reference)
"""Boundary loss kernel for Trainium2 (8 NeuronCores, SPMD).

loss = mean(sigmoid(pred) * EDT(target)) for pred/target [4,1,512,512].

Algorithm (per core: one sample s = c//2, one 256-row half j2 = c%2):
  Vertical pass: EXACT 1D city-block distance per column via two
  tensor_tensor_scan ops (fwd: state=min(state+1, nbt); bwd fused with the
  min against the fwd result) on a transposed [w, h] layout. Cross-column
  scan leakage is bounded >= 3 at all output rows (2-row halo), so it can
  never beat a certified dist <= sqrt(8) and needs no reset.
  TensorE transposes [w,h] -> [h,w]; squares land via tt.mult from PSUM.
  Horizontal pass: windowed (+-2) min over g^2 + dx^2 using 2x-rate
  tensor_tensor mins and 4x-rate tensor_scalar adds (the baseline's
  1x-rate scalar_tensor_tensor ops are gone).
  Tail: sqrt on ScalarE, sigmoid*dist product on VectorE, and the
  per-partition row sums via a plain VectorE tensor_reduce (avoids the
  ~0.9us DVE accumulator-read drain). Host sums the [128,2] partials.

  Exactness certificate (host, ~free): if every pixel lies in the 5x5 box
  dilation of the mask, the windowed-horizontal result equals the exact
  EDT. Random ~50% masks always pass; otherwise fall back to exact numpy.

Work split: VectorE scans column groups 0-1 and runs the horizontal
min-chains; GpSimd scans groups 2-3, does memsets and the PSUM squares;
ScalarE does sigmoid + sqrt; TensorE transposes + reduction matmuls; DMAs
are issued from Scalar/Sync/Vector so their ~1.2us issue costs overlap.
"""

import sys

sys.path.insert(0, "/opt/trn_rl_repo")

import numpy as np
import ml_dtypes

BIG = 16384.0
PAD = 2
B, H, W = 4, 512, 512
HALF = 256
HALO = HALF + 2 * PAD  # 260
GW = 4 * HALO  # 1040, free width of the [w, h] layout
MW = W + 2 * PAD  # 516, phase-B row width incl pads

_compiled = None


def _build_bass():
    import concourse.bacc as bacc
    import concourse.tile as tile
    from concourse import mybir

    nc = bacc.Bacc(None)
    dt = mybir.dt
    Alu = mybir.AluOpType
    Act = mybir.ActivationFunctionType

    # Host-packed inputs:
    #   nbt[p, t*HALO + h] = BIG*(1-mask) at column w = t*128+p, halo row h
    #   pred[p, j*512 + x] = logits at row r0 + j*128 + p, col x (bf16)
    nbt_d = nc.dram_tensor("nbt", [128, GW], dt.bfloat16, kind="ExternalInput")
    pred_d = nc.dram_tensor("pred", [128, 2 * W], dt.bfloat16, kind="ExternalInput")
    out_d = nc.dram_tensor("out", [128, 2 * W], dt.bfloat16, kind="ExternalOutput")
    ident_d = nc.inline_tensor(
        np.eye(128, dtype=ml_dtypes.bfloat16), name="ident_const"
    )

    with tile.TileContext(nc) as tc:
        with (
            tc.tile_pool(name="sb", bufs=1) as sb,
            tc.tile_pool(name="ps", bufs=1, space="PSUM") as ps,
        ):
            nbt = sb.tile([128, 4, HALO], dt.bfloat16)
            pred_sb = sb.tile([128, 2 * W], dt.bfloat16)
            ident = sb.tile([128, 128], dt.bfloat16)
            sig = sb.tile([128, 2 * W], dt.bfloat16)
            sig2 = sb.tile([128, 2 * W], dt.bfloat16)
            pv1 = sb.tile([128, 4, HALF], dt.bfloat16)
            pv2 = sb.tile([128, 4, HALF], dt.bfloat16)
            sv1 = sb.tile([128, 4, HALF], dt.bfloat16)
            sv2 = sb.tile([128, 4, HALF], dt.bfloat16)
            mv1 = sb.tile([128, 4, HALF], dt.bfloat16)
            acc = sb.tile([128, 4, HALF], dt.bfloat16)
            m2 = [sb.tile([128, MW], dt.bfloat16, name=f"m2_{j}") for j in range(2)]
            p1 = [sb.tile([128, W], dt.bfloat16, name=f"p1_{j}") for j in range(2)]
            p2 = [sb.tile([128, W], dt.bfloat16, name=f"p2_{j}") for j in range(2)]
            s1 = [sb.tile([128, W], dt.bfloat16, name=f"s1_{j}") for j in range(2)]
            s2 = [sb.tile([128, W], dt.bfloat16, name=f"s2_{j}") for j in range(2)]
            m1 = [sb.tile([128, W], dt.bfloat16, name=f"m1_{j}") for j in range(2)]
            d2 = [sb.tile([128, W], dt.bfloat16, name=f"d2_{j}") for j in range(2)]
            sd = [sb.tile([128, W], dt.bfloat16, name=f"sd_{j}") for j in range(2)]
            outp = sb.tile([128, 2 * W], dt.bfloat16)
            pt = [ps.tile([128, W], dt.bfloat16, name=f"pt_{j}") for j in range(2)]
            wj = ps.tile([128, 128], dt.bfloat16)

            # --- DMAs: ScalarE issues nbt (this engine/queue pairing
            # measured the fastest transfer), Sync takes pred, GpSimd the
            # identity.
            nc.scalar.dma_start(
                out=nbt[:], in_=nbt_d[:].rearrange("p (t h) -> p t h", t=4)
            )
            nc.scalar.dma_start(out=pred_sb[:], in_=pred_d[:])
            nc.sync.dma_start(out=ident[:], in_=ident_d[:])

            # GpSimd: phase-B pad columns during the DMA wait.
            for j in range(2):
                nc.gpsimd.memset(m2[j][:, 0:PAD], BIG)
                nc.gpsimd.memset(m2[j][:, PAD + W : MW], BIG)

            # TensorE warm-up (p-state ramp) on the identity.
            for _ in range(2):
                nc.tensor.transpose(out=wj[:], in_=ident[:], identity=ident[:])

            # --- Vertical pass on VectorE: windowed min with the SQUARED dy
            # penalties applied directly, so acc = g^2 with no squaring step:
            #   acc = min(nbt_0, 1 + min(nbt+-1), 4 + min(nbt+-2))
            # The final min is split per row-half so TensorE can start the
            # j0 transposes while j1's half still computes.
            P = PAD
            tt, ts = nc.vector.tensor_tensor, nc.vector.tensor_scalar
            tt(out=pv1[:], in0=nbt[:, :, P - 1 : P - 1 + HALF],
               in1=nbt[:, :, P + 1 : P + 1 + HALF], op=Alu.min)
            tt(out=pv2[:], in0=nbt[:, :, P - 2 : P - 2 + HALF],
               in1=nbt[:, :, P + 2 : P + 2 + HALF], op=Alu.min)
            ts(out=sv1[:], in0=pv1[:], scalar1=1.0, scalar2=None, op0=Alu.add)
            ts(out=sv2[:], in0=pv2[:], scalar1=4.0, scalar2=None, op0=Alu.add)
            tt(out=mv1[:], in0=nbt[:, :, P : P + HALF], in1=sv1[:], op=Alu.min)
            for j in range(2):
                tt(out=acc[:, :, j * 128 : (j + 1) * 128],
                   in0=mv1[:, :, j * 128 : (j + 1) * 128],
                   in1=sv2[:, :, j * 128 : (j + 1) * 128], op=Alu.min)

            # Transpose [w,h] -> [h,w], j0 blocks first.
            for j in range(2):
                for t in range(4):
                    nc.tensor.transpose(
                        out=pt[j][:, t * 128 : (t + 1) * 128],
                        in_=acc[:, t, j * 128 : (j + 1) * 128],
                        identity=ident[:],
                    )

            # Evacuate PSUM: VectorE copies j0 (2x), ScalarE copies j1 in
            # parallel after its sigmoid work.
            nc.vector.tensor_copy(out=m2[0][:, PAD : PAD + W], in_=pt[0][:])
            nc.scalar.activation(out=sig[:], in_=pred_sb[:], func=Act.Sigmoid)
            nc.scalar.activation(out=sig2[:], in_=sig[:], func=Act.Square)
            nc.scalar.copy(out=m2[1][:, PAD : PAD + W], in_=pt[1][:])

            # --- Horizontal windowed min on VectorE, j=0 staged ahead:
            #     d2 = min(g2_0, 1+min(g2+-1), 4+min(g2+-2)); sd = sig^2*d2;
            #     sqrt(sd) = sig*dist lands in the output tile via ScalarE.
            for j in range(2):
                tt(out=p1[j][:], in0=m2[j][:, 1 : 1 + W],
                   in1=m2[j][:, 3 : 3 + W], op=Alu.min)
                tt(out=p2[j][:], in0=m2[j][:, 0:W],
                   in1=m2[j][:, 4 : 4 + W], op=Alu.min)
                ts(out=s1[j][:], in0=p1[j][:], scalar1=1.0, scalar2=None,
                   op0=Alu.add)
                ts(out=s2[j][:], in0=p2[j][:], scalar1=4.0, scalar2=None,
                   op0=Alu.add)
                tt(out=m1[j][:], in0=m2[j][:, 2 : 2 + W], in1=s1[j][:],
                   op=Alu.min)
                tt(out=d2[j][:], in0=m1[j][:], in1=s2[j][:], op=Alu.min)
                nc.vector.tensor_tensor(
                    out=sd[j][:], in0=sig2[:, j * W : (j + 1) * W],
                    in1=d2[j][:], op=Alu.mult,
                )
                nc.scalar.activation(
                    out=outp[:, j * W : (j + 1) * W], in_=sd[j][:],
                    func=Act.Sqrt,
                )
                nc.sync.dma_start(
                    out=out_d[:, j * W : (j + 1) * W],
                    in_=outp[:, j * W : (j + 1) * W],
                )

    nc.finalize()
    return nc


def _exact_loss_numpy(pred, target):
    """Exact fallback, matching reference.py semantics."""
    mask = target[:, 0].astype(np.float32)
    b, h, w = mask.shape
    big = np.float32(h + w)
    rows = np.arange(h, dtype=np.float32)[None, :, None]
    fg = mask > 0
    last = np.maximum.accumulate(np.where(fg, rows, -big), axis=1)
    nxt = np.minimum.accumulate(np.where(fg, rows, 3 * big)[:, ::-1], axis=1)[:, ::-1]
    g = np.minimum(np.minimum(rows - last, nxt - rows), big)
    g2 = (g * g).astype(np.float32)
    cols = np.arange(w, dtype=np.float32)
    diff2 = (cols[:, None] - cols[None, :]) ** 2
    dist = np.empty((b, h, w), np.float32)
    for bi in range(b):
        for r0 in range(0, h, 64):
            blk = g2[bi, r0 : r0 + 64]
            dist[bi, r0 : r0 + 64] = np.sqrt(
                (diff2[None, :, :] + blk[:, None, :]).min(-1)
            )
    has_fg = fg.any(axis=(1, 2))
    dist = np.where(has_fg[:, None, None], dist, 0.0)
    p = 1.0 / (1.0 + np.exp(-pred[:, 0].astype(np.float64)))
    return np.float32((p * dist).mean())


def _cert_ok(target):
    """Host-side exactness certificate: the +-2-window horizontal pass (after
    an exact vertical pass) is exact iff every pixel of each foreground-bearing
    sample lies in the 5x5 box dilation of the mask."""
    fg = target[:, 0] > 0  # [B, H, W]

    def dil1d(a, axis):
        out = a.copy()
        for s in (1, 2):
            hi = [slice(None)] * a.ndim
            lo = [slice(None)] * a.ndim
            hi[axis] = slice(s, None)
            lo[axis] = slice(None, -s)
            np.logical_or(out[tuple(hi)], a[tuple(lo)], out=out[tuple(hi)])
            np.logical_or(out[tuple(lo)], a[tuple(hi)], out=out[tuple(lo)])
        return out

    cov = dil1d(dil1d(fg, 1), 2).all(axis=(1, 2))  # [B]
    has_fg = fg.any(axis=(1, 2))
    return bool(np.all(cov | ~has_fg))


def _prep_in_maps(pred, target):
    bf16 = ml_dtypes.bfloat16
    mask = (target[:, 0] > 0).astype(np.float32)  # [B, H, W]
    in_maps = []
    for c in range(8):
        s, j2 = c // 2, c % 2
        r0 = j2 * HALF
        halo = np.zeros((HALO, W), np.float32)
        lo, hi = r0 - PAD, r0 + HALF + PAD
        slo, shi = max(lo, 0), min(hi, H)
        halo[slo - lo : shi - lo] = mask[s, slo:shi]
        # nbt[p, t*HALO + h] for column w = t*128+p
        nbt_wh = (BIG * (1.0 - halo)).T  # [W, HALO]
        nbt = np.ascontiguousarray(
            nbt_wh.reshape(4, 128, HALO).transpose(1, 0, 2).reshape(128, GW)
        ).astype(bf16)
        # pred[p, j*512 + x] for row r0 + j*128 + p (bf16)
        ph = pred[s, 0, r0 : r0 + HALF, :].astype(np.float32)
        predh = np.ascontiguousarray(
            ph.reshape(2, 128, W).transpose(1, 0, 2).reshape(128, 2 * W)
        ).astype(bf16)
        in_maps.append({"nbt": nbt, "pred": predh})
    return in_maps


def kernel_with_results(pred, target, trace=False):
    """Returns (loss, BassKernelResults)."""
    global _compiled
    from concourse.bass_utils import run_bass_kernel_spmd

    if _compiled is None:
        _compiled = _build_bass()
    nc = _compiled

    in_maps = _prep_in_maps(pred, target)
    bkr = run_bass_kernel_spmd(nc, in_maps, core_ids=list(range(8)), trace=trace)

    if not _cert_ok(target):
        # Windowed EDT not certified exact for this input; fall back.
        return _exact_loss_numpy(pred, target), bkr

    has_fg = (target[:, 0] > 0).any(axis=(1, 2))  # [B]
    total = np.float64(0.0)
    for c in range(8):
        if not has_fg[c // 2]:
            continue
        out = bkr.results[c]["out"]  # [128, 1024] bf16 sig*dist terms
        total += out.astype(np.float64).sum()

    loss = np.array(total / (B * 1 * H * W), dtype=np.float32)
    return loss, bkr


def kernel(pred, target):
    loss, _ = kernel_with_results(pred, target)
    return loss
